# revision 17
# baseline (speedup 1.0000x reference)
"""AttentionUpscaling Trainium2 kernel.

Strategy (8 NeuronCores):
  - Pure data parallelism over batch (4) x query-half (2): each core owns one
    (batch, q-half) shard of the L x L attention matmul (the ~97 GFLOP that
    dominate this problem).
  - Host side (sharding prep): bilinear 2x upsample (exact jax semantics via a
    sparse banded matrix), unfold of the high-frequency residual, fp8e4m3
    quantization (attn scaled by 2^14 so row-stochastic weights stay in the
    normal range), and per-core relayout so the DoubleRow matmul reads both
    operands with unit-stride DMA.
  - Device side (SPMD bass/Tile program, same NEFF on all 8 cores):
    rec[q, d] = sum_m attnT[m, q] * hf[m, d] in fp8e4 with
    perf_mode=DoubleRow (2 fp8 weights per PE cell, K=256 per matmul,
    0.5 cycles/row): attn is the stationary operand ([128, 2, 128] tiles,
    one weight load per 256-row contraction chunk serves the full 768-wide
    moving hf), hf SBUF-resident, attn streamed one q-tile (512 KB) per DMA
    double-buffered, fp32 PSUM accumulation, DVE copyback, HWDGE DMA out.
  - Host side (gather): rescale (2^-14), overlap-add fold + overlap-count
    normalization + base image add, then stitch the two q-halves per batch.
"""

import os

import numpy as np

# ---------------------------------------------------------------- constants
B, C = 4, 3
HH = 512          # HR height/width
HL = 256          # LR height/width
K = 16            # HR patch size
S = 8             # HR stride
NH = (HH - K) // S + 1          # 63 patches per axis
L = NH * NH                     # 3969 patches
CKK = C * K * K                 # 768
NPH = 32                        # patch-rows per core (ph 0..31 / 31..62)
LQ = NPH * NH                   # 2016 q rows per core
LQP = 2048                      # padded q rows (16 x 128)
MP = 4096                       # padded contraction dim (16 x 256)
N_CORES = 8
NQT = LQP // 128                # 16 q-tiles of 128
NT = MP // 256                  # 16 DoubleRow contraction chunks of 256
D_SPLIT = 384                   # matmul moving free-dim (2 x 384 = 768)
ATTN_SCALE = 16384.0            # 2^14: lifts ~2.5e-4 attn weights into the
                                # fp8e4m3 normal range; divided back out on host

LAST_RESULT = None              # BassKernelResults of the most recent run


# ------------------------------------------------------------- host helpers
def _bilinear_up_matrix() -> np.ndarray:
    """U (512, 256): exact jax.image.resize 'bilinear' 256->512 upsample.

    Half-pixel centers: src(o) = o/2 - 0.25; triangle weights, renormalized
    at the edges (matches jax's scale_and_translate for scale 2 upsampling).
    """
    U = np.zeros((HH, HL), np.float32)
    for o in range(HH):
        src = o / 2.0 - 0.25
        i0 = int(np.floor(src))
        f = src - i0
        w = {i0: 1.0 - f, i0 + 1: f}
        valid = {i: wi for i, wi in w.items() if 0 <= i < HL and wi > 0}
        tot = sum(valid.values())
        for i, wi in valid.items():
            U[o, i] = wi / tot
    return U


_U = _bilinear_up_matrix()


def _upsample2(x: np.ndarray) -> np.ndarray:
    """(..., 256, 256) -> (..., 512, 512) bilinear, exact jax semantics."""
    lead = x.shape[:-2]
    xf = x.reshape((-1, HL, HL)).astype(np.float32)
    y = np.einsum("yi,nij,xj->nyx", _U, xf, _U, optimize=True)
    return y.reshape(lead + (HH, HH)).astype(np.float32)


def _unfold_hf(x_hr_b: np.ndarray, blur_hr_b: np.ndarray) -> np.ndarray:
    """hf (L, CKK): unfold(x_hr - blur_hr, k=16, s=8), m=(ph,pw), d=(c,i,j)."""
    d = (x_hr_b - blur_hr_b).astype(np.float32)          # (C, 512, 512)
    win = np.lib.stride_tricks.sliding_window_view(d, (K, K), axis=(1, 2))
    win = win[:, ::S, ::S]                                # (C, 63, 63, 16, 16)
    return np.ascontiguousarray(
        win.transpose(1, 2, 0, 3, 4).reshape(L, CKK))


def _fold(cols: np.ndarray) -> np.ndarray:
    """cols (B, CKK, L) -> overlap-add (B, C, 512, 512) (reference col2im)."""
    c6 = cols.reshape(B, C, K, K, NH, NH)
    out = np.zeros((B, C, HH, HH), np.float32)
    for i in range(K):
        for j in range(K):
            out[:, :, i:i + S * NH:S, j:j + S * NH:S] += c6[:, :, i, j]
    return out


_NORM = None


def _norm_map() -> np.ndarray:
    global _NORM
    if _NORM is None:
        _NORM = _fold(np.ones((B, CKK, L), np.float32))
        _NORM = np.maximum(_NORM, 1e-8)
    return _NORM


def _fp8(x: np.ndarray) -> np.ndarray:
    import ml_dtypes
    return x.astype(ml_dtypes.float8_e4m3)


# ------------------------------------------------------------ device kernel
_NC = None


def _build_nc(qts_per_ctx=NQT):
    """SPMD bass program: rec = attnT.T @ hf, fp8e4 DoubleRow matmuls.

    DRAM layouts (host-prepared, unit-stride DMA):
      attn_dr [NQT*128, NT*256] fp8: row qt*128+p, col t*256 + i*128 + q
        holds attnT[m = t*256 + i*128 + p, qt*128 + q] * ATTN_SCALE.
      hf_dr [128, NT*2*CKK] fp8: row p, col t*1536 + i*768 + d holds
        hf[m = t*256 + i*128 + p, d].
      rec [LQP, CKK] fp32 output.
    """
    import bass_rust
    import concourse.bass as bass
    import concourse.mybir as mybir
    from concourse.tile import TileContext
    from concourse.vector_clock import ScopedClock

    # Walrus in this build rejects ctrl instructions carrying >2 sem waits;
    # Tile's exit drain waits on every live semaphore.  Split those waits
    # across single-wait drain instructions.
    def _drain_and_barrier(self, tick_clock, wait_clock):
        nc = self.nc
        drain_inst = nc.sync.drain()
        wait_clock.add_sem_waits(
            drain_inst.ins, ScopedClock({None: tick_clock.global_clock}))
        si = drain_inst.ins.sync_info
        waits = list(si.on_wait)
        if len(waits) > 1:
            drain_inst.ins.sync_info = bass_rust.SyncInfo(
                on_update=list(si.on_update), on_wait=waits[:1])
            for w in waits[1:]:
                d2 = nc.sync.drain()
                d2.ins.sync_info = bass_rust.SyncInfo(on_update=[], on_wait=[w])
        nc.all_engine_barrier()
        popped = nc._tile_sem_poison_stack.pop()
        assert popped is self._sem_poison
        nc.clear_and_free_semaphores(list(self.sems.allocated().values()))
        nc.all_engine_barrier()

    TileContext._drain_and_barrier = _drain_and_barrier

    # Engine sem-name prefix per engine type, for the self-wait post-pass.
    _ENG_SEM = {
        mybir.EngineType.PE: "PE_",
        mybir.EngineType.DVE: "DVE_",
        mybir.EngineType.Activation: "Activation_",
        mybir.EngineType.SP: "SP_",
        mybir.EngineType.Pool: "Pool_",
    }

    prelude_nops = []    # (engine, nop) last-resort wait carriers, per context

    def _split_excess_waits(nc):
        """Walrus in this build caps sem waits per instruction (1 for DMA,
        2 otherwise).  Two legal rewrites bring Tile's output under the cap:
          - drop self-engine waits (WAW on a reused slot): engines complete
            in order, so an earlier same-engine producer is already done;
          - hoist remaining excess waits onto the nearest *preceding*
            same-engine instruction with spare capacity — the sequencer
            executes waits in program order, so waiting earlier is strictly
            more conservative.  (Producers of hoisted waits are tile-slot
            reuses >= one full band older, so no deadlock is possible.)
        """
        import bass_rust as _br

        prelude_by_name = {i.ins.name: i.ins for _, i in prelude_nops}

        def cap(inst):
            # Empirically this walrus accepts at most ONE sem wait per
            # instruction across every struct we hit (DMA, ACT, LW/matmul,
            # ctrl drain).
            return 1

        def set_waits(inst, waits):
            si = inst.sync_info
            ups = list(si.on_update) if si else []
            inst.sync_info = _br.SyncInfo(on_update=ups, on_wait=waits)

        def merge_wait(inst, w):
            """Add wait w to inst, merging same-sem waits by max value."""
            si = inst.sync_info
            waits = list(si.on_wait) if si else []
            for i, ex in enumerate(waits):
                if ex.ant_name == w.ant_name:
                    if w.wait_value > ex.wait_value:
                        waits[i] = w
                    set_waits(inst, waits)
                    return
            set_waits(inst, waits + [w])

        for bb in nc.main_func.blocks:
            streams = {}            # engine -> prior instructions, in order
            bb_preludes = {}        # engine -> prelude nops IN THIS BB only
            for inst in bb.instructions:
                stream = streams.setdefault(inst.engine, [])
                if inst.name in prelude_by_name:
                    bb_preludes.setdefault(inst.engine, []).append(inst)
                    stream.append(inst)
                    continue
                si = inst.sync_info
                if si is None:
                    stream.append(inst)
                    continue
                waits = list(si.on_wait)
                if len(waits) <= cap(inst):
                    stream.append(inst)
                    continue
                # 1) drop self-engine waits (in-order engines: an earlier
                #    same-engine producer has completed by issue time)
                pfx = _ENG_SEM.get(inst.engine)
                waits = [w for w in waits
                         if not (pfx and w.ant_name.startswith(pfx))]
                # 1b) a WAR wait on the ACT dummy-read is implied by the WAR
                #     wait on the ACT-issued output DMA (same sequencer,
                #     in-order: dummy completed before the DMA was issued)
                if (len(waits) > cap(inst)
                        and any(w.ant_name.startswith("DMAHW") for w in waits)):
                    waits = [w for w in waits
                             if not w.ant_name.startswith("Activation_")]
                if len(waits) > cap(inst):
                    # keep one wait (prefer the DMA-lane RAW for DMAs), hoist
                    # the rest onto earlier same-engine instructions — waits
                    # execute in sequencer program order, so hoisting is
                    # strictly more conservative.  Producers of hoisted waits
                    # are tile-slot reuses from >= 2 pipeline stages earlier,
                    # so a bounded backward hoist cannot deadlock.
                    if type(inst).__name__ == "InstDMACopy":
                        keep = ([w for w in waits if w.ant_name.startswith("DMAHW")]
                                or waits)[:1]
                    else:
                        keep = waits[:1]
                    hoist = [w for w in waits if w not in keep]
                    for w in hoist:
                        placed = False
                        if not placed:
                            for prior in reversed(stream[-50:]):
                                psi = prior.sync_info
                                pw = list(psi.on_wait) if psi else []
                                if len(pw) < cap(prior):
                                    set_waits(prior, pw + [w])
                                    placed = True
                                    break
                        if not placed:
                            # last resort: prelude nop on this engine (they
                            # sit at the head of this context's stream)
                            for pn in bb_preludes.get(inst.engine, []):
                                psi = pn.sync_info
                                pw = list(psi.on_wait) if psi else []
                                same = [x for x in pw if x.ant_name == w.ant_name]
                                if same or len(pw) < 1:
                                    merge_wait(pn, w)
                                    placed = True
                                    break
                        assert placed, (
                            f"{inst.name}: no carrier for {w.ant_name}")
                    waits = keep
                assert len(waits) <= cap(inst), (
                    f"{inst.name}: still {len(waits)} waits")
                set_waits(inst, waits)
                stream.append(inst)

    dt = mybir.dt
    f32 = dt.float32
    f8 = dt.float8e4
    DR = mybir.MatmulPerfMode.DoubleRow

    nc = bass.Bass(target_bir_lowering=False)
    attn_dr = nc.dram_tensor("attn_dr", [NQT * 128, NT * 256], f8,
                             kind="ExternalInput")
    hf_dr = nc.dram_tensor("hf_dr", [128, NT * 2 * CKK], f8,
                           kind="ExternalInput")
    rec = nc.dram_tensor("rec", [LQP, CKK], f32, kind="ExternalOutput")

    ds = CKK // 2

    # Single TileContext; every attn q-tile and every output staging tile
    # gets its own SBUF slot (no slot reuse => no WAR waits on any DMA, the
    # class of waits that previously needed carrier nops and could deadlock
    # when the scheduler moved those free nops).  SBUF/partition: attn 64K +
    # hf 24K + rec staging ~49K = ~137K of ~208K usable.
    with TileContext(nc) as tc:
        with (
            tc.tile_pool(name="hfp", bufs=1) as hfp,
            tc.tile_pool(name="attp", bufs=1) as attp,
            tc.tile_pool(name="recp", bufs=1) as recp,
            tc.tile_pool(name="dmyp", bufs=1) as dmyp,
            tc.tile_pool(name="psp", bufs=2, space="PSUM") as psp,
        ):
            for eng_name, eng in (("tensor", nc.tensor),
                                  ("vector", nc.vector),
                                  ("scalar", nc.scalar)):
                for i in range(8):
                    prelude_nops.append(
                        (eng.engine,
                         eng.nop(hint=f"prelude_{eng_name}_{i}")))

            hf_sb = hfp.tile([128, NT, 2, CKK], f8, tag="hf")
            nc.sync.dma_start(hf_sb[:, :, :, :], hf_dr[:, :])
            at_tiles = []
            for qt in range(NQT):
                at = attp.tile([128, NT, 2, 128], f8, tag=f"at{qt}")
                nc.sync.dma_start(
                    at[:, :, :, :],
                    attn_dr[qt * 128:(qt + 1) * 128, :])
                at_tiles.append(at)

            for qt in range(NQT):
                at = at_tiles[qt]
                p0 = psp.tile([128, D_SPLIT], f32, tag="p0")
                p1 = psp.tile([128, D_SPLIT], f32, tag="p1")
                for t in range(NT):
                    lhs = at[:, t, :, :]
                    nc.tensor.matmul(
                        p0[:, 0:ds], lhs, hf_sb[:, t, :, 0:ds],
                        start=(t == 0), stop=(t == NT - 1),
                        perf_mode=DR)
                    nc.tensor.matmul(
                        p1[:, 0:ds], lhs, hf_sb[:, t, :, ds:CKK],
                        start=(t == 0), stop=(t == NT - 1),
                        perf_mode=DR)
                ro0 = recp.tile([128, D_SPLIT], f32, tag=f"rec{qt}_0")
                ro1 = recp.tile([128, D_SPLIT], f32, tag=f"rec{qt}_1")
                nc.vector.tensor_copy(ro0[:, 0:ds], p0[:, 0:ds])
                nc.vector.tensor_copy(ro1[:, 0:ds], p1[:, 0:ds])
                # ACT observes the DVE copies via this cheap read, so
                # the ACT-issued output DMAs need no extra DVE wait
                # of their own (Tile elides observed ticks).
                dmy0 = dmyp.tile([128, 1], f32, tag=f"dmy{qt}_0")
                dmy1 = dmyp.tile([128, 1], f32, tag=f"dmy{qt}_1")
                nc.scalar.copy(dmy0[:], ro0[:, 0:1])
                nc.scalar.copy(dmy1[:], ro1[:, 0:1])
                q0 = qt * 128
                nc.scalar.dma_start(rec[q0:q0 + 128, 0:ds],
                                    ro0[:, 0:ds])
                nc.scalar.dma_start(rec[q0:q0 + 128, ds:CKK],
                                    ro1[:, 0:ds])
    _split_excess_waits(nc)
    return nc


def _get_nc():
    global _NC
    if _NC is None:
        _NC = _build_nc()
    return _NC


# ---------------------------------------------------------------- benchmark
def bench(in_maps, iters: int = 10):
    """Steady-state per-execution wall time of the compiled NEFF.

    Re-implements bass2jax.run_bass_via_pjrt's jit/shard_map wrapping, but
    device_puts the inputs once and dispatches `iters` executions
    asynchronously, blocking only at the end — so per-call axon RPC latency
    pipelines away and (total / iters) approaches the on-device time.
    """
    import time

    import jax
    import numpy as np
    from jax.experimental.shard_map import shard_map
    from jax.sharding import Mesh, NamedSharding, PartitionSpec

    import concourse.bass2jax as bass2jax
    import concourse.mybir as mybir

    nc = _get_nc()
    bass2jax.install_neuronx_cc_hook()

    partition_name = (nc.partition_id_tensor.name
                      if nc.partition_id_tensor else None)
    in_names, out_names, out_avals, zero_outs = [], [], [], []
    for alloc in nc.m.functions[0].allocations:
        if not isinstance(alloc, mybir.MemoryLocationSet):
            continue
        name = alloc.memorylocations[0].name
        if alloc.kind == "ExternalInput":
            if name != partition_name:
                in_names.append(name)
        elif alloc.kind == "ExternalOutput":
            shape = tuple(alloc.tensor_shape)
            dtype = mybir.dt.np(alloc.dtype)
            out_names.append(name)
            out_avals.append(jax.core.ShapedArray(shape, dtype))
            zero_outs.append(np.zeros(shape, dtype))
    n_params = len(in_names)
    n_outs = len(out_avals)
    all_names = in_names + out_names
    if partition_name is not None:
        all_names = all_names + [partition_name]
    donate = tuple(range(n_params, n_params + n_outs))

    def _body(*args):
        operands = list(args)
        if partition_name is not None:
            operands.append(bass2jax.partition_id_tensor())
        outs = bass2jax._bass_exec_p.bind(
            *operands,
            out_avals=tuple(out_avals),
            in_names=tuple(all_names),
            out_names=tuple(out_names),
            lowering_input_output_aliases=(),
            sim_require_finite=True,
            sim_require_nnan=True,
            nc=nc,
        )
        return tuple(outs)

    devices = jax.devices()[:N_CORES]
    mesh = Mesh(np.asarray(devices), ("core",))
    sh = NamedSharding(mesh, PartitionSpec("core"))
    sharded = jax.jit(
        shard_map(_body, mesh=mesh,
                  in_specs=(PartitionSpec("core"),) * (n_params + n_outs),
                  out_specs=(PartitionSpec("core"),) * n_outs,
                  check_rep=False),
        donate_argnums=donate, keep_unused=True)

    concat_in = [
        np.concatenate([np.asarray(in_maps[c][nm]) for c in range(N_CORES)], 0)
        for nm in in_names
    ]
    dev_in = [jax.device_put(a, sh) for a in concat_in]
    mk_zeros = lambda: [
        jax.device_put(np.zeros((N_CORES * z.shape[0], *z.shape[1:]), z.dtype), sh)
        for z in zero_outs
    ]

    warm = sharded(*dev_in, *mk_zeros())
    jax.block_until_ready(warm)

    zbufs = [mk_zeros() for _ in range(iters)]
    outs = []
    t0 = time.perf_counter()
    for i in range(iters):
        outs.append(sharded(*dev_in, *zbufs[i]))
    jax.block_until_ready(outs)
    t1 = time.perf_counter()
    per_call_ns = (t1 - t0) / iters * 1e9
    return per_call_ns, warm


# ------------------------------------------------------------------- kernel
def _prepare(x_hr, x_lr_inpainted, attn_map, x_lr_blurred):
    """Host sharding prep: upsample, unfold, fp8 quantize + relayout."""
    x_hr = np.asarray(x_hr, np.float32)
    x_lr_inpainted = np.asarray(x_lr_inpainted, np.float32)
    attn_map = np.asarray(attn_map, np.float32)
    x_lr_blurred = np.asarray(x_lr_blurred, np.float32)

    blur_hr = _upsample2(x_lr_blurred)                    # (B, C, 512, 512)
    base = _upsample2(x_lr_inpainted)                     # (B, C, 512, 512)

    q_starts = (0, L - LQ)                                # 0 and 1953
    in_maps = []
    hf_cache = {}
    for core in range(N_CORES):
        b, half = core // 2, core % 2
        if b not in hf_cache:
            hfp = np.zeros((MP, CKK), np.float32)
            hfp[:L] = _unfold_hf(x_hr[b], blur_hr[b])
            # [m, d] -> [p, t, i, d] with m = t*256 + i*128 + p
            hq = hfp.reshape(NT, 2, 128, CKK).transpose(2, 0, 1, 3)
            hf_cache[b] = np.ascontiguousarray(
                _fp8(hq).reshape(128, NT * 2 * CKK))
        q0 = q_starts[half]
        ap = np.zeros((LQP, MP), np.float32)
        ap[:LQ, :L] = attn_map[b, 0, q0:q0 + LQ, :] * ATTN_SCALE
        # [qi, m] -> [qt, p, t, i, q] with qi = qt*128 + q, m = t*256+i*128+p
        a5 = ap.reshape(NQT, 128, NT, 2, 128).transpose(0, 4, 2, 3, 1)
        at = np.ascontiguousarray(_fp8(a5).reshape(NQT * 128, NT * 256))
        in_maps.append({"attn_dr": at, "hf_dr": hf_cache[b]})
    return in_maps, base


def _finish(per_core_rec, base):
    """Gather: rescale, stitch q-halves, fold, normalize, add base."""
    inv = 1.0 / ATTN_SCALE
    cols = np.empty((B, CKK, L), np.float32)
    for b in range(B):
        rec_a = per_core_rec[2 * b]                       # (2048, 768)
        rec_b = per_core_rec[2 * b + 1]
        cols[b, :, :LQ] = rec_a[:LQ].T * inv
        cols[b, :, LQ:] = rec_b[2 * LQ - L:LQ].T * inv
    img = _fold(cols)
    out = base + img / _norm_map()
    return out.astype(np.float32)


def kernel(x_hr, x_lr_inpainted, attn_map, x_lr_blurred):
    global LAST_RESULT
    from concourse.bass_utils import run_bass_kernel_spmd

    in_maps, base = _prepare(x_hr, x_lr_inpainted, attn_map, x_lr_blurred)
    nc = _get_nc()
    trace = bool(os.environ.get("KERNEL_TRACE"))
    res = run_bass_kernel_spmd(nc, in_maps, list(range(N_CORES)), trace=trace)
    LAST_RESULT = res
    return _finish([res.results[c]["rec"] for c in range(N_CORES)], base)


# revision 22
# speedup vs baseline: 1.0396x; 1.0396x over previous
"""AttentionUpscaling Trainium2 kernel.

Strategy (8 NeuronCores):
  - Pure data parallelism over batch (4) x query-half (2): each core owns one
    (batch, q-half) shard of the L x L attention matmul (the ~97 GFLOP that
    dominate this problem).
  - Host side (sharding prep): bilinear 2x upsample (exact jax semantics via a
    sparse banded matrix), unfold of the high-frequency residual, fp8e4m3
    quantization (attn scaled by 2^14 so row-stochastic weights stay in the
    normal range), and per-core relayout so the DoubleRow matmul reads both
    operands with unit-stride DMA.
  - Device side (SPMD bass/Tile program, same NEFF on all 8 cores):
    rec[q, d] = sum_m attnT[m, q] * hf[m, d] in fp8e4 with
    perf_mode=DoubleRow (2 fp8 weights per PE cell, K=256 per matmul,
    0.5 cycles/row): attn is the stationary operand ([128, 2, 128] tiles,
    one weight load per 256-row contraction chunk serves the full 768-wide
    moving hf), hf SBUF-resident, attn streamed one q-tile (512 KB) per DMA
    double-buffered, fp32 PSUM accumulation, DVE copyback, HWDGE DMA out.
  - Host side (gather): rescale (2^-14), overlap-add fold + overlap-count
    normalization + base image add, then stitch the two q-halves per batch.
"""

import os

import numpy as np

# ---------------------------------------------------------------- constants
B, C = 4, 3
HH = 512          # HR height/width
HL = 256          # LR height/width
K = 16            # HR patch size
S = 8             # HR stride
NH = (HH - K) // S + 1          # 63 patches per axis
L = NH * NH                     # 3969 patches
CKK = C * K * K                 # 768
NPH = 32                        # patch-rows per core (ph 0..31 / 31..62)
LQ = NPH * NH                   # 2016 q rows per core
LQP = 2048                      # padded q rows (16 x 128)
MP = 4096                       # padded contraction dim (16 x 256)
N_CORES = 8
NQT = LQP // 128                # 16 q-tiles of 128
NT = MP // 256                  # 16 DoubleRow contraction chunks of 256
D_SPLIT = 384                   # matmul moving free-dim (2 x 384 = 768)
ATTN_SCALE = 16384.0            # 2^14: lifts ~2.5e-4 attn weights into the
                                # fp8e4m3 normal range; divided back out on host

LAST_RESULT = None              # BassKernelResults of the most recent run


# ------------------------------------------------------------- host helpers
def _bilinear_up_matrix() -> np.ndarray:
    """U (512, 256): exact jax.image.resize 'bilinear' 256->512 upsample.

    Half-pixel centers: src(o) = o/2 - 0.25; triangle weights, renormalized
    at the edges (matches jax's scale_and_translate for scale 2 upsampling).
    """
    U = np.zeros((HH, HL), np.float32)
    for o in range(HH):
        src = o / 2.0 - 0.25
        i0 = int(np.floor(src))
        f = src - i0
        w = {i0: 1.0 - f, i0 + 1: f}
        valid = {i: wi for i, wi in w.items() if 0 <= i < HL and wi > 0}
        tot = sum(valid.values())
        for i, wi in valid.items():
            U[o, i] = wi / tot
    return U


_U = _bilinear_up_matrix()


def _upsample2(x: np.ndarray) -> np.ndarray:
    """(..., 256, 256) -> (..., 512, 512) bilinear, exact jax semantics."""
    lead = x.shape[:-2]
    xf = x.reshape((-1, HL, HL)).astype(np.float32)
    y = np.einsum("yi,nij,xj->nyx", _U, xf, _U, optimize=True)
    return y.reshape(lead + (HH, HH)).astype(np.float32)


def _unfold_hf(x_hr_b: np.ndarray, blur_hr_b: np.ndarray) -> np.ndarray:
    """hf (L, CKK): unfold(x_hr - blur_hr, k=16, s=8), m=(ph,pw), d=(c,i,j)."""
    d = (x_hr_b - blur_hr_b).astype(np.float32)          # (C, 512, 512)
    win = np.lib.stride_tricks.sliding_window_view(d, (K, K), axis=(1, 2))
    win = win[:, ::S, ::S]                                # (C, 63, 63, 16, 16)
    return np.ascontiguousarray(
        win.transpose(1, 2, 0, 3, 4).reshape(L, CKK))


def _fold(cols: np.ndarray) -> np.ndarray:
    """cols (B, CKK, L) -> overlap-add (B, C, 512, 512) (reference col2im)."""
    c6 = cols.reshape(B, C, K, K, NH, NH)
    out = np.zeros((B, C, HH, HH), np.float32)
    for i in range(K):
        for j in range(K):
            out[:, :, i:i + S * NH:S, j:j + S * NH:S] += c6[:, :, i, j]
    return out


_NORM = None


def _norm_map() -> np.ndarray:
    global _NORM
    if _NORM is None:
        _NORM = _fold(np.ones((B, CKK, L), np.float32))
        _NORM = np.maximum(_NORM, 1e-8)
    return _NORM


def _fp8(x: np.ndarray) -> np.ndarray:
    import ml_dtypes
    return x.astype(ml_dtypes.float8_e4m3)


# ------------------------------------------------------------ device kernel
_NC = None


def _build_nc(qts_per_ctx=NQT):
    """SPMD bass program: rec = attnT.T @ hf, fp8e4 DoubleRow matmuls.

    DRAM layouts (host-prepared, unit-stride DMA):
      attn_dr [NQT*128, NT*256] fp8: row qt*128+p, col t*256 + i*128 + q
        holds attnT[m = t*256 + i*128 + p, qt*128 + q] * ATTN_SCALE.
      hf_dr [128, NT*2*CKK] fp8: row p, col t*1536 + i*768 + d holds
        hf[m = t*256 + i*128 + p, d].
      rec [LQP, CKK] fp32 output.
    """
    import bass_rust
    import concourse.bass as bass
    import concourse.mybir as mybir
    from concourse.tile import TileContext
    from concourse.vector_clock import ScopedClock

    # Walrus in this build rejects ctrl instructions carrying >2 sem waits;
    # Tile's exit drain waits on every live semaphore.  Split those waits
    # across single-wait drain instructions.
    def _drain_and_barrier(self, tick_clock, wait_clock):
        nc = self.nc
        drain_inst = nc.sync.drain()
        wait_clock.add_sem_waits(
            drain_inst.ins, ScopedClock({None: tick_clock.global_clock}))
        si = drain_inst.ins.sync_info
        waits = list(si.on_wait)
        if len(waits) > 1:
            drain_inst.ins.sync_info = bass_rust.SyncInfo(
                on_update=list(si.on_update), on_wait=waits[:1])
            for w in waits[1:]:
                d2 = nc.sync.drain()
                d2.ins.sync_info = bass_rust.SyncInfo(on_update=[], on_wait=[w])
        nc.all_engine_barrier()
        popped = nc._tile_sem_poison_stack.pop()
        assert popped is self._sem_poison
        nc.clear_and_free_semaphores(list(self.sems.allocated().values()))
        nc.all_engine_barrier()

    TileContext._drain_and_barrier = _drain_and_barrier

    # Engine sem-name prefix per engine type, for the self-wait post-pass.
    _ENG_SEM = {
        mybir.EngineType.PE: "PE_",
        mybir.EngineType.DVE: "DVE_",
        mybir.EngineType.Activation: "Activation_",
        mybir.EngineType.SP: "SP_",
        mybir.EngineType.Pool: "Pool_",
    }

    prelude_nops = []    # (engine, nop) last-resort wait carriers, per context

    def _split_excess_waits(nc):
        """Walrus in this build caps sem waits per instruction (1 for DMA,
        2 otherwise).  Two legal rewrites bring Tile's output under the cap:
          - drop self-engine waits (WAW on a reused slot): engines complete
            in order, so an earlier same-engine producer is already done;
          - hoist remaining excess waits onto the nearest *preceding*
            same-engine instruction with spare capacity — the sequencer
            executes waits in program order, so waiting earlier is strictly
            more conservative.  (Producers of hoisted waits are tile-slot
            reuses >= one full band older, so no deadlock is possible.)
        """
        import bass_rust as _br

        prelude_by_name = {i.ins.name: i.ins for _, i in prelude_nops}

        def cap(inst):
            # Empirically this walrus accepts at most ONE sem wait per
            # instruction across every struct we hit (DMA, ACT, LW/matmul,
            # ctrl drain).
            return 1

        def set_waits(inst, waits):
            si = inst.sync_info
            ups = list(si.on_update) if si else []
            inst.sync_info = _br.SyncInfo(on_update=ups, on_wait=waits)

        def merge_wait(inst, w):
            """Add wait w to inst, merging same-sem waits by max value."""
            si = inst.sync_info
            waits = list(si.on_wait) if si else []
            for i, ex in enumerate(waits):
                if ex.ant_name == w.ant_name:
                    if w.wait_value > ex.wait_value:
                        waits[i] = w
                    set_waits(inst, waits)
                    return
            set_waits(inst, waits + [w])

        for bb in nc.main_func.blocks:
            streams = {}            # engine -> prior instructions, in order
            bb_preludes = {}        # engine -> prelude nops IN THIS BB only
            for inst in bb.instructions:
                stream = streams.setdefault(inst.engine, [])
                if inst.name in prelude_by_name:
                    bb_preludes.setdefault(inst.engine, []).append(inst)
                    stream.append(inst)
                    continue
                si = inst.sync_info
                if si is None:
                    stream.append(inst)
                    continue
                waits = list(si.on_wait)
                if len(waits) <= cap(inst):
                    stream.append(inst)
                    continue
                # 1) drop self-engine waits (in-order engines: an earlier
                #    same-engine producer has completed by issue time)
                pfx = _ENG_SEM.get(inst.engine)
                waits = [w for w in waits
                         if not (pfx and w.ant_name.startswith(pfx))]
                # 1b) a WAR wait on the ACT dummy-read is implied by the WAR
                #     wait on the ACT-issued output DMA (same sequencer,
                #     in-order: dummy completed before the DMA was issued)
                if (len(waits) > cap(inst)
                        and any(w.ant_name.startswith("DMAHW") for w in waits)):
                    waits = [w for w in waits
                             if not w.ant_name.startswith("Activation_")]
                if len(waits) > cap(inst):
                    # keep one wait (prefer the DMA-lane RAW for DMAs), hoist
                    # the rest onto earlier same-engine instructions — waits
                    # execute in sequencer program order, so hoisting is
                    # strictly more conservative.  Producers of hoisted waits
                    # are tile-slot reuses from >= 2 pipeline stages earlier,
                    # so a bounded backward hoist cannot deadlock.
                    if type(inst).__name__ == "InstDMACopy":
                        keep = ([w for w in waits if w.ant_name.startswith("DMAHW")]
                                or waits)[:1]
                    else:
                        keep = waits[:1]
                    hoist = [w for w in waits if w not in keep]
                    for w in hoist:
                        placed = False
                        if not placed:
                            for prior in reversed(stream[-50:]):
                                psi = prior.sync_info
                                pw = list(psi.on_wait) if psi else []
                                if len(pw) < cap(prior):
                                    set_waits(prior, pw + [w])
                                    placed = True
                                    break
                        if not placed:
                            # last resort: prelude nop on this engine (they
                            # sit at the head of this context's stream)
                            for pn in bb_preludes.get(inst.engine, []):
                                psi = pn.sync_info
                                pw = list(psi.on_wait) if psi else []
                                same = [x for x in pw if x.ant_name == w.ant_name]
                                if same or len(pw) < 1:
                                    merge_wait(pn, w)
                                    placed = True
                                    break
                        assert placed, (
                            f"{inst.name}: no carrier for {w.ant_name}")
                    waits = keep
                assert len(waits) <= cap(inst), (
                    f"{inst.name}: still {len(waits)} waits")
                set_waits(inst, waits)
                stream.append(inst)

    dt = mybir.dt
    f32 = dt.float32
    f8 = dt.float8e4
    DR = mybir.MatmulPerfMode.DoubleRow

    nc = bass.Bass(target_bir_lowering=False)
    attn_dr = nc.dram_tensor("attn_dr", [NQT * 128, NT * 256], f8,
                             kind="ExternalInput")
    hf_dr = nc.dram_tensor("hf_dr", [128, NT * 2 * CKK], f8,
                           kind="ExternalInput")
    rec = nc.dram_tensor("rec", [LQP, CKK], f32, kind="ExternalOutput")

    ds = CKK // 2

    # Single TileContext; every attn q-tile and every output staging tile
    # gets its own SBUF slot (no slot reuse => no WAR waits on any DMA, the
    # class of waits that previously needed carrier nops and could deadlock
    # when the scheduler moved those free nops).  SBUF/partition: attn 64K +
    # hf 24K + rec staging ~49K = ~137K of ~208K usable.
    with TileContext(nc) as tc:
        with (
            tc.tile_pool(name="hfp", bufs=1) as hfp,
            tc.tile_pool(name="attp", bufs=1) as attp,
            tc.tile_pool(name="recp", bufs=1) as recp,
            tc.tile_pool(name="dmyp", bufs=1) as dmyp,
            tc.tile_pool(name="psp", bufs=2, space="PSUM") as psp,
        ):
            for eng_name, eng in (("tensor", nc.tensor),
                                  ("vector", nc.vector),
                                  ("scalar", nc.scalar)):
                for i in range(8):
                    prelude_nops.append(
                        (eng.engine,
                         eng.nop(hint=f"prelude_{eng_name}_{i}")))

            hf_sb = hfp.tile([128, NT, 2, CKK], f8, tag="hf")
            nc.sync.dma_start(hf_sb[:, :, :, :], hf_dr[:, :])
            at_tiles = []
            for qt in range(NQT):
                at = attp.tile([128, NT, 2, 128], f8, tag=f"at{qt}")
                nc.sync.dma_start(
                    at[:, :, :, :],
                    attn_dr[qt * 128:(qt + 1) * 128, :])
                at_tiles.append(at)

            for qt in range(NQT):
                at = at_tiles[qt]
                p0 = psp.tile([128, D_SPLIT], f32, tag="p0")
                p1 = psp.tile([128, D_SPLIT], f32, tag="p1")
                for t in range(NT):
                    lhs = at[:, t, :, :]
                    nc.tensor.matmul(
                        p0[:, 0:ds], lhs, hf_sb[:, t, :, 0:ds],
                        start=(t == 0), stop=(t == NT - 1),
                        perf_mode=DR)
                    nc.tensor.matmul(
                        p1[:, 0:ds], lhs, hf_sb[:, t, :, ds:CKK],
                        start=(t == 0), stop=(t == NT - 1),
                        perf_mode=DR)
                ro0 = recp.tile([128, D_SPLIT], f32, tag=f"rec{qt}_0")
                ro1 = recp.tile([128, D_SPLIT], f32, tag=f"rec{qt}_1")
                nc.vector.tensor_copy(ro0[:, 0:ds], p0[:, 0:ds])
                nc.vector.tensor_copy(ro1[:, 0:ds], p1[:, 0:ds])
                # ACT observes the DVE copies via this cheap read, so
                # the ACT-issued output DMAs need no extra DVE wait
                # of their own (Tile elides observed ticks).
                dmy0 = dmyp.tile([128, 1], f32, tag=f"dmy{qt}_0")
                dmy1 = dmyp.tile([128, 1], f32, tag=f"dmy{qt}_1")
                nc.scalar.copy(dmy0[:], ro0[:, 0:1])
                nc.scalar.copy(dmy1[:], ro1[:, 0:1])
                q0 = qt * 128
                nc.scalar.dma_start(rec[q0:q0 + 128, 0:ds],
                                    ro0[:, 0:ds])
                nc.scalar.dma_start(rec[q0:q0 + 128, ds:CKK],
                                    ro1[:, 0:ds])
    _split_excess_waits(nc)
    return nc


def _get_nc():
    global _NC
    if _NC is None:
        _NC = _build_nc()
    return _NC


# ---------------------------------------------------------------- benchmark
def bench(in_maps, iters: int = 10):
    """Steady-state per-execution wall time of the compiled NEFF.

    Re-implements bass2jax.run_bass_via_pjrt's jit/shard_map wrapping, but
    device_puts the inputs once and dispatches `iters` executions
    asynchronously, blocking only at the end — so per-call axon RPC latency
    pipelines away and (total / iters) approaches the on-device time.
    """
    import time

    import jax
    import numpy as np
    from jax.experimental.shard_map import shard_map
    from jax.sharding import Mesh, NamedSharding, PartitionSpec

    import concourse.bass2jax as bass2jax
    import concourse.mybir as mybir

    nc = _get_nc()
    bass2jax.install_neuronx_cc_hook()

    partition_name = (nc.partition_id_tensor.name
                      if nc.partition_id_tensor else None)
    in_names, out_names, out_avals, zero_outs = [], [], [], []
    for alloc in nc.m.functions[0].allocations:
        if not isinstance(alloc, mybir.MemoryLocationSet):
            continue
        name = alloc.memorylocations[0].name
        if alloc.kind == "ExternalInput":
            if name != partition_name:
                in_names.append(name)
        elif alloc.kind == "ExternalOutput":
            shape = tuple(alloc.tensor_shape)
            dtype = mybir.dt.np(alloc.dtype)
            out_names.append(name)
            out_avals.append(jax.core.ShapedArray(shape, dtype))
            zero_outs.append(np.zeros(shape, dtype))
    n_params = len(in_names)
    n_outs = len(out_avals)
    all_names = in_names + out_names
    if partition_name is not None:
        all_names = all_names + [partition_name]
    donate = tuple(range(n_params, n_params + n_outs))

    def _body(*args):
        operands = list(args)
        if partition_name is not None:
            operands.append(bass2jax.partition_id_tensor())
        outs = bass2jax._bass_exec_p.bind(
            *operands,
            out_avals=tuple(out_avals),
            in_names=tuple(all_names),
            out_names=tuple(out_names),
            lowering_input_output_aliases=(),
            sim_require_finite=True,
            sim_require_nnan=True,
            nc=nc,
        )
        return tuple(outs)

    devices = jax.devices()[:N_CORES]
    mesh = Mesh(np.asarray(devices), ("core",))
    sh = NamedSharding(mesh, PartitionSpec("core"))

    sharded = jax.jit(
        shard_map(_body, mesh=mesh,
                  in_specs=(PartitionSpec("core"),) * (n_params + n_outs),
                  out_specs=(PartitionSpec("core"),) * n_outs,
                  check_rep=False),
        donate_argnums=donate, keep_unused=True)

    concat_in = [
        np.concatenate([np.asarray(in_maps[c][nm]) for c in range(N_CORES)], 0)
        for nm in in_names
    ]
    dev_in = [jax.device_put(a, sh) for a in concat_in]
    mk_zeros = lambda: [
        jax.device_put(np.zeros((N_CORES * z.shape[0], *z.shape[1:]), z.dtype), sh)
        for z in zero_outs
    ]

    warm = sharded(*dev_in, *mk_zeros())
    jax.block_until_ready(warm)

    zbufs = [mk_zeros() for _ in range(iters)]
    jax.block_until_ready(zbufs)   # finish H2D staging before the clock
    outs = []
    t0 = time.perf_counter()
    for i in range(iters):
        outs.append(sharded(*dev_in, *zbufs[i]))
    jax.block_until_ready(outs)
    t1 = time.perf_counter()
    per_call_ns = (t1 - t0) / iters * 1e9
    return per_call_ns, warm


# ------------------------------------------------------------------- kernel
def _prepare(x_hr, x_lr_inpainted, attn_map, x_lr_blurred):
    """Host sharding prep: upsample, unfold, fp8 quantize + relayout."""
    x_hr = np.asarray(x_hr, np.float32)
    x_lr_inpainted = np.asarray(x_lr_inpainted, np.float32)
    attn_map = np.asarray(attn_map, np.float32)
    x_lr_blurred = np.asarray(x_lr_blurred, np.float32)

    blur_hr = _upsample2(x_lr_blurred)                    # (B, C, 512, 512)
    base = _upsample2(x_lr_inpainted)                     # (B, C, 512, 512)

    q_starts = (0, L - LQ)                                # 0 and 1953
    in_maps = []
    hf_cache = {}
    for core in range(N_CORES):
        b, half = core // 2, core % 2
        if b not in hf_cache:
            hfp = np.zeros((MP, CKK), np.float32)
            hfp[:L] = _unfold_hf(x_hr[b], blur_hr[b])
            # [m, d] -> [p, t, i, d] with m = t*256 + i*128 + p
            hq = hfp.reshape(NT, 2, 128, CKK).transpose(2, 0, 1, 3)
            hf_cache[b] = np.ascontiguousarray(
                _fp8(hq).reshape(128, NT * 2 * CKK))
        q0 = q_starts[half]
        ap = np.zeros((LQP, MP), np.float32)
        ap[:LQ, :L] = attn_map[b, 0, q0:q0 + LQ, :] * ATTN_SCALE
        # [qi, m] -> [qt, p, t, i, q] with qi = qt*128 + q, m = t*256+i*128+p
        a5 = ap.reshape(NQT, 128, NT, 2, 128).transpose(0, 4, 2, 3, 1)
        at = np.ascontiguousarray(_fp8(a5).reshape(NQT * 128, NT * 256))
        in_maps.append({"attn_dr": at, "hf_dr": hf_cache[b]})
    return in_maps, base


def _finish(per_core_rec, base):
    """Gather: rescale, stitch q-halves, fold, normalize, add base."""
    inv = 1.0 / ATTN_SCALE
    cols = np.empty((B, CKK, L), np.float32)
    for b in range(B):
        rec_a = per_core_rec[2 * b]                       # (2048, 768)
        rec_b = per_core_rec[2 * b + 1]
        cols[b, :, :LQ] = rec_a[:LQ].T * inv
        cols[b, :, LQ:] = rec_b[2 * LQ - L:LQ].T * inv
    img = _fold(cols)
    out = base + img / _norm_map()
    return out.astype(np.float32)


def kernel(x_hr, x_lr_inpainted, attn_map, x_lr_blurred):
    global LAST_RESULT
    from concourse.bass_utils import run_bass_kernel_spmd

    in_maps, base = _prepare(x_hr, x_lr_inpainted, attn_map, x_lr_blurred)
    nc = _get_nc()
    trace = bool(os.environ.get("KERNEL_TRACE"))
    res = run_bass_kernel_spmd(nc, in_maps, list(range(N_CORES)), trace=trace)
    LAST_RESULT = res
    return _finish([res.results[c]["rec"] for c in range(N_CORES)], base)


# revision 29
# speedup vs baseline: 213.7299x; 205.5810x over previous
"""AttentionUpscaling Trainium2 kernel.

Strategy (8 NeuronCores):
  - Pure data parallelism over batch (4) x query-half (2): each core owns one
    (batch, q-half) shard of the L x L attention matmul (the ~97 GFLOP that
    dominate this problem).
  - Host side (sharding prep): bilinear 2x upsample (exact jax semantics via a
    sparse banded matrix), unfold of the high-frequency residual, fp8e4m3
    quantization (attn scaled by 2^14 so row-stochastic weights stay in the
    normal range), and per-core relayout so the DoubleRow matmul reads both
    operands with unit-stride DMA.
  - Device side (SPMD bass/Tile program, same NEFF on all 8 cores):
    rec[q, d] = sum_m attnT[m, q] * hf[m, d] in fp8e4 with
    perf_mode=DoubleRow (2 fp8 weights per PE cell, K=256 per matmul,
    0.5 cycles/row): attn is the stationary operand ([128, 2, 128] tiles,
    one weight load per 256-row contraction chunk serves the full 768-wide
    moving hf), hf SBUF-resident, attn streamed one q-tile (512 KB) per DMA
    double-buffered, fp32 PSUM accumulation, DVE copyback, HWDGE DMA out.
  - Host side (gather): rescale (2^-14), overlap-add fold + overlap-count
    normalization + base image add, then stitch the two q-halves per batch.
"""

import os

import numpy as np

# ---------------------------------------------------------------- constants
B, C = 4, 3
HH = 512          # HR height/width
HL = 256          # LR height/width
K = 16            # HR patch size
S = 8             # HR stride
NH = (HH - K) // S + 1          # 63 patches per axis
L = NH * NH                     # 3969 patches
CKK = C * K * K                 # 768
NPH = 32                        # patch-rows per core (ph 0..31 / 31..62)
LQ = NPH * NH                   # 2016 q rows per core
LQP = 2048                      # padded q rows (16 x 128)
MP = 4096                       # padded contraction dim (16 x 256)
N_CORES = 8
NQT = LQP // 128                # 16 q-tiles of 128
NT = MP // 256                  # 16 DoubleRow contraction chunks of 256
D_SPLIT = 384                   # matmul moving free-dim (2 x 384 = 768)
SWI = True                      # DoubleRowSwInterleave (host-interleaved
                                # weights, contiguous LDWEIGHTS reads)
ATTN_SCALE = 16384.0            # 2^14: lifts ~2.5e-4 attn weights into the
                                # fp8e4m3 normal range; divided back out on host

LAST_RESULT = None              # BassKernelResults of the most recent run


# ------------------------------------------------------------- host helpers
def _bilinear_up_matrix() -> np.ndarray:
    """U (512, 256): exact jax.image.resize 'bilinear' 256->512 upsample.

    Half-pixel centers: src(o) = o/2 - 0.25; triangle weights, renormalized
    at the edges (matches jax's scale_and_translate for scale 2 upsampling).
    """
    U = np.zeros((HH, HL), np.float32)
    for o in range(HH):
        src = o / 2.0 - 0.25
        i0 = int(np.floor(src))
        f = src - i0
        w = {i0: 1.0 - f, i0 + 1: f}
        valid = {i: wi for i, wi in w.items() if 0 <= i < HL and wi > 0}
        tot = sum(valid.values())
        for i, wi in valid.items():
            U[o, i] = wi / tot
    return U


_U = _bilinear_up_matrix()


def _upsample2(x: np.ndarray) -> np.ndarray:
    """(..., 256, 256) -> (..., 512, 512) bilinear, exact jax semantics."""
    lead = x.shape[:-2]
    xf = x.reshape((-1, HL, HL)).astype(np.float32)
    y = np.einsum("yi,nij,xj->nyx", _U, xf, _U, optimize=True)
    return y.reshape(lead + (HH, HH)).astype(np.float32)


def _unfold_hf(x_hr_b: np.ndarray, blur_hr_b: np.ndarray) -> np.ndarray:
    """hf (L, CKK): unfold(x_hr - blur_hr, k=16, s=8), m=(ph,pw), d=(c,i,j)."""
    d = (x_hr_b - blur_hr_b).astype(np.float32)          # (C, 512, 512)
    win = np.lib.stride_tricks.sliding_window_view(d, (K, K), axis=(1, 2))
    win = win[:, ::S, ::S]                                # (C, 63, 63, 16, 16)
    return np.ascontiguousarray(
        win.transpose(1, 2, 0, 3, 4).reshape(L, CKK))


def _fold(cols: np.ndarray) -> np.ndarray:
    """cols (B, CKK, L) -> overlap-add (B, C, 512, 512) (reference col2im)."""
    c6 = cols.reshape(B, C, K, K, NH, NH)
    out = np.zeros((B, C, HH, HH), np.float32)
    for i in range(K):
        for j in range(K):
            out[:, :, i:i + S * NH:S, j:j + S * NH:S] += c6[:, :, i, j]
    return out


_NORM = None


def _norm_map() -> np.ndarray:
    global _NORM
    if _NORM is None:
        _NORM = _fold(np.ones((B, CKK, L), np.float32))
        _NORM = np.maximum(_NORM, 1e-8)
    return _NORM


def _fp8(x: np.ndarray) -> np.ndarray:
    import ml_dtypes
    return x.astype(ml_dtypes.float8_e4m3)


# ------------------------------------------------------------ device kernel
_NC = None


def _build_nc(reps=1):
    """SPMD bass program: rec = attnT.T @ hf, fp8e4 DoubleRow matmuls.

    reps > 1 repeats the whole computation (one TileContext per rep) so a
    bench can amortize the ~11 ms per-dispatch axon overhead and recover the
    true per-execution device time from (t_reps - t_1) / (reps - 1).

    DRAM layouts (host-prepared, unit-stride DMA):
      attn_dr [NQT*128, NT*256] fp8: row qt*128+p, col t*256 + i*128 + q
        holds attnT[m = t*256 + i*128 + p, qt*128 + q] * ATTN_SCALE.
      hf_dr [128, NT*2*CKK] fp8: row p, col t*1536 + i*768 + d holds
        hf[m = t*256 + i*128 + p, d].
      rec [LQP, CKK] fp32 output.
    """
    import bass_rust
    import concourse.bass as bass
    import concourse.mybir as mybir
    from concourse.tile import TileContext
    from concourse.vector_clock import ScopedClock

    # Walrus in this build rejects ctrl instructions carrying >2 sem waits;
    # Tile's exit drain waits on every live semaphore.  Split those waits
    # across single-wait drain instructions.
    def _drain_and_barrier(self, tick_clock, wait_clock):
        nc = self.nc
        drain_inst = nc.sync.drain()
        wait_clock.add_sem_waits(
            drain_inst.ins, ScopedClock({None: tick_clock.global_clock}))
        si = drain_inst.ins.sync_info
        waits = list(si.on_wait)
        if len(waits) > 1:
            drain_inst.ins.sync_info = bass_rust.SyncInfo(
                on_update=list(si.on_update), on_wait=waits[:1])
            for w in waits[1:]:
                d2 = nc.sync.drain()
                d2.ins.sync_info = bass_rust.SyncInfo(on_update=[], on_wait=[w])
        nc.all_engine_barrier()
        popped = nc._tile_sem_poison_stack.pop()
        assert popped is self._sem_poison
        nc.clear_and_free_semaphores(list(self.sems.allocated().values()))
        nc.all_engine_barrier()

    TileContext._drain_and_barrier = _drain_and_barrier

    # Engine sem-name prefix per engine type, for the self-wait post-pass.
    _ENG_SEM = {
        mybir.EngineType.PE: "PE_",
        mybir.EngineType.DVE: "DVE_",
        mybir.EngineType.Activation: "Activation_",
        mybir.EngineType.SP: "SP_",
        mybir.EngineType.Pool: "Pool_",
    }

    prelude_nops = []    # (engine, nop) last-resort wait carriers, per context

    def _split_excess_waits(nc):
        """Walrus in this build caps sem waits per instruction (1 for DMA,
        2 otherwise).  Two legal rewrites bring Tile's output under the cap:
          - drop self-engine waits (WAW on a reused slot): engines complete
            in order, so an earlier same-engine producer is already done;
          - hoist remaining excess waits onto the nearest *preceding*
            same-engine instruction with spare capacity — the sequencer
            executes waits in program order, so waiting earlier is strictly
            more conservative.  (Producers of hoisted waits are tile-slot
            reuses >= one full band older, so no deadlock is possible.)
        """
        import bass_rust as _br

        prelude_by_name = {i.ins.name: i.ins for _, i in prelude_nops}

        def cap(inst):
            # Empirically this walrus accepts at most ONE sem wait per
            # instruction across every struct we hit (DMA, ACT, LW/matmul,
            # ctrl drain).
            return 1

        def set_waits(inst, waits):
            si = inst.sync_info
            ups = list(si.on_update) if si else []
            inst.sync_info = _br.SyncInfo(on_update=ups, on_wait=waits)

        def merge_wait(inst, w):
            """Add wait w to inst, merging same-sem waits by max value."""
            si = inst.sync_info
            waits = list(si.on_wait) if si else []
            for i, ex in enumerate(waits):
                if ex.ant_name == w.ant_name:
                    if w.wait_value > ex.wait_value:
                        waits[i] = w
                    set_waits(inst, waits)
                    return
            set_waits(inst, waits + [w])

        for bb in nc.main_func.blocks:
            streams = {}            # engine -> prior instructions, in order
            bb_preludes = {}        # engine -> prelude nops IN THIS BB only
            for inst in bb.instructions:
                stream = streams.setdefault(inst.engine, [])
                if inst.name in prelude_by_name:
                    bb_preludes.setdefault(inst.engine, []).append(inst)
                    stream.append(inst)
                    continue
                si = inst.sync_info
                if si is None:
                    stream.append(inst)
                    continue
                waits = list(si.on_wait)
                if len(waits) <= cap(inst):
                    stream.append(inst)
                    continue
                # 1) drop self-engine waits (in-order engines: an earlier
                #    same-engine producer has completed by issue time)
                pfx = _ENG_SEM.get(inst.engine)
                waits = [w for w in waits
                         if not (pfx and w.ant_name.startswith(pfx))]
                # 1b) a WAR wait on the ACT dummy-read is implied by the WAR
                #     wait on the ACT-issued output DMA (same sequencer,
                #     in-order: dummy completed before the DMA was issued)
                if (len(waits) > cap(inst)
                        and any(w.ant_name.startswith("DMAHW") for w in waits)):
                    waits = [w for w in waits
                             if not w.ant_name.startswith("Activation_")]
                if len(waits) > cap(inst):
                    # keep one wait (prefer the DMA-lane RAW for DMAs), hoist
                    # the rest onto earlier same-engine instructions — waits
                    # execute in sequencer program order, so hoisting is
                    # strictly more conservative.  Producers of hoisted waits
                    # are tile-slot reuses from >= 2 pipeline stages earlier,
                    # so a bounded backward hoist cannot deadlock.
                    if type(inst).__name__ == "InstDMACopy":
                        keep = ([w for w in waits if w.ant_name.startswith("DMAHW")]
                                or waits)[:1]
                    else:
                        keep = waits[:1]
                    hoist = [w for w in waits if w not in keep]
                    for w in hoist:
                        placed = False
                        if not placed:
                            for prior in reversed(stream[-50:]):
                                psi = prior.sync_info
                                pw = list(psi.on_wait) if psi else []
                                if len(pw) < cap(prior):
                                    set_waits(prior, pw + [w])
                                    placed = True
                                    break
                        if not placed:
                            # last resort: prelude nop on this engine (they
                            # sit at the head of this context's stream)
                            for pn in bb_preludes.get(inst.engine, []):
                                psi = pn.sync_info
                                pw = list(psi.on_wait) if psi else []
                                same = [x for x in pw if x.ant_name == w.ant_name]
                                if same or len(pw) < 1:
                                    merge_wait(pn, w)
                                    placed = True
                                    break
                        assert placed, (
                            f"{inst.name}: no carrier for {w.ant_name}")
                    waits = keep
                assert len(waits) <= cap(inst), (
                    f"{inst.name}: still {len(waits)} waits")
                set_waits(inst, waits)
                stream.append(inst)

    dt = mybir.dt
    f32 = dt.float32
    f8 = dt.float8e4
    DR = (mybir.MatmulPerfMode.DoubleRowSwInterleave if SWI
          else mybir.MatmulPerfMode.DoubleRow)

    nc = bass.Bass(target_bir_lowering=False)
    attn_dr = nc.dram_tensor("attn_dr", [NQT * 128, NT * 256], f8,
                             kind="ExternalInput")
    hf_dr = nc.dram_tensor("hf_dr", [128, NT * 2 * CKK], f8,
                           kind="ExternalInput")
    rec = nc.dram_tensor("rec", [LQP, CKK], f32, kind="ExternalOutput")

    ds = CKK // 2

    # Single TileContext; every attn q-tile and every output staging tile
    # gets its own SBUF slot (no slot reuse => no WAR waits on any DMA, the
    # class of waits that previously needed carrier nops and could deadlock
    # when the scheduler moved those free nops).  SBUF/partition: attn 64K +
    # hf 24K + rec staging ~49K = ~137K of ~208K usable.
    for _rep in range(reps):
      with TileContext(nc) as tc:
        with (
            tc.tile_pool(name="hfp", bufs=1) as hfp,
            tc.tile_pool(name="attp", bufs=1) as attp,
            tc.tile_pool(name="recp", bufs=1) as recp,
            tc.tile_pool(name="dmyp", bufs=1) as dmyp,
            tc.tile_pool(name="psp", bufs=2, space="PSUM") as psp,
        ):
            for eng_name, eng in (("tensor", nc.tensor),
                                  ("vector", nc.vector),
                                  ("scalar", nc.scalar)):
                for i in range(8):
                    prelude_nops.append(
                        (eng.engine,
                         eng.nop(hint=f"prelude_{eng_name}_{i}")))

            # qt0's attn tile first (the first matmul RAW-waits on it), then
            # hf in 4 ~790 KB chunks (qt0's t-loop consumes them in arrival
            # order), then the remaining attn tiles prefetch behind.
            at_tiles = [attp.tile([128, NT, 2, 128], f8, tag="at0",
                                  name="at0")]
            nc.sync.dma_start(at_tiles[0][:, :, :, :], attn_dr[0:128, :])
            TG = 4                       # t-chunks per hf DMA
            hf_groups = []
            for g in range(NT // TG):
                hfg = hfp.tile([128, TG, 2, CKK], f8, tag=f"hf{g}")
                nc.sync.dma_start(
                    hfg[:, :, :, :],
                    hf_dr[:, g * TG * 2 * CKK:(g + 1) * TG * 2 * CKK])
                hf_groups.append(hfg)
            hf_tiles = [hf_groups[t // TG][:, t % TG, :, :] for t in range(NT)]
            for qt in range(1, NQT):
                at = attp.tile([128, NT, 2, 128], f8, tag=f"at{qt}")
                nc.sync.dma_start(
                    at[:, :, :, :],
                    attn_dr[qt * 128:(qt + 1) * 128, :])
                at_tiles.append(at)

            for qt in range(NQT):
                at = at_tiles[qt]
                p0 = psp.tile([128, D_SPLIT], f32, tag="p0")
                p1 = psp.tile([128, D_SPLIT], f32, tag="p1")
                for t in range(NT):
                    lhs = at[:, t, :, :]
                    nc.tensor.matmul(
                        p0[:, 0:ds], lhs, hf_tiles[t][:, :, 0:ds],
                        start=(t == 0), stop=(t == NT - 1),
                        perf_mode=DR)
                    nc.tensor.matmul(
                        p1[:, 0:ds], lhs, hf_tiles[t][:, :, ds:CKK],
                        start=(t == 0), stop=(t == NT - 1),
                        perf_mode=DR)
                ro0 = recp.tile([128, D_SPLIT], f32, tag=f"rec{qt}_0")
                ro1 = recp.tile([128, D_SPLIT], f32, tag=f"rec{qt}_1")
                nc.vector.tensor_copy(ro0[:, 0:ds], p0[:, 0:ds])
                nc.vector.tensor_copy(ro1[:, 0:ds], p1[:, 0:ds])
                # ACT observes the DVE copies via this cheap read, so
                # the ACT-issued output DMAs need no extra DVE wait
                # of their own (Tile elides observed ticks).
                dmy0 = dmyp.tile([128, 1], f32, tag=f"dmy{qt}_0")
                dmy1 = dmyp.tile([128, 1], f32, tag=f"dmy{qt}_1")
                nc.scalar.copy(dmy0[:], ro0[:, 0:1])
                nc.scalar.copy(dmy1[:], ro1[:, 0:1])
                q0 = qt * 128
                nc.scalar.dma_start(rec[q0:q0 + 128, 0:ds],
                                    ro0[:, 0:ds])
                nc.scalar.dma_start(rec[q0:q0 + 128, ds:CKK],
                                    ro1[:, 0:ds])
    _split_excess_waits(nc)
    return nc


def _get_nc():
    global _NC
    if _NC is None:
        _NC = _build_nc()
    return _NC


# ---------------------------------------------------------------- benchmark
def bench(in_maps, iters: int = 10, nc=None):
    """Mean per-dispatch wall time of the compiled NEFF (axon-RPC bound).

    Device_puts the inputs once and dispatches `iters` executions
    asynchronously, blocking only at the end.  Under this axon client each
    dispatch carries ~11 ms of RPC/dispatch overhead, so use bench_device()
    (reps differencing) for the true on-device time.
    """
    import time

    import jax
    import numpy as np
    from jax.experimental.shard_map import shard_map
    from jax.sharding import Mesh, NamedSharding, PartitionSpec

    import concourse.bass2jax as bass2jax
    import concourse.mybir as mybir

    if nc is None:
        nc = _get_nc()
    bass2jax.install_neuronx_cc_hook()

    partition_name = (nc.partition_id_tensor.name
                      if nc.partition_id_tensor else None)
    in_names, out_names, out_avals, zero_outs = [], [], [], []
    for alloc in nc.m.functions[0].allocations:
        if not isinstance(alloc, mybir.MemoryLocationSet):
            continue
        name = alloc.memorylocations[0].name
        if alloc.kind == "ExternalInput":
            if name != partition_name:
                in_names.append(name)
        elif alloc.kind == "ExternalOutput":
            shape = tuple(alloc.tensor_shape)
            dtype = mybir.dt.np(alloc.dtype)
            out_names.append(name)
            out_avals.append(jax.core.ShapedArray(shape, dtype))
            zero_outs.append(np.zeros(shape, dtype))
    n_params = len(in_names)
    n_outs = len(out_avals)
    all_names = in_names + out_names
    if partition_name is not None:
        all_names = all_names + [partition_name]
    donate = tuple(range(n_params, n_params + n_outs))

    def _body(*args):
        operands = list(args)
        if partition_name is not None:
            operands.append(bass2jax.partition_id_tensor())
        outs = bass2jax._bass_exec_p.bind(
            *operands,
            out_avals=tuple(out_avals),
            in_names=tuple(all_names),
            out_names=tuple(out_names),
            lowering_input_output_aliases=(),
            sim_require_finite=True,
            sim_require_nnan=True,
            nc=nc,
        )
        return tuple(outs)

    devices = jax.devices()[:N_CORES]
    mesh = Mesh(np.asarray(devices), ("core",))
    sh = NamedSharding(mesh, PartitionSpec("core"))

    sharded = jax.jit(
        shard_map(_body, mesh=mesh,
                  in_specs=(PartitionSpec("core"),) * (n_params + n_outs),
                  out_specs=(PartitionSpec("core"),) * n_outs,
                  check_rep=False),
        donate_argnums=donate, keep_unused=True)

    concat_in = [
        np.concatenate([np.asarray(in_maps[c][nm]) for c in range(N_CORES)], 0)
        for nm in in_names
    ]
    dev_in = [jax.device_put(a, sh) for a in concat_in]
    mk_zeros = lambda: [
        jax.device_put(np.zeros((N_CORES * z.shape[0], *z.shape[1:]), z.dtype), sh)
        for z in zero_outs
    ]

    warm = sharded(*dev_in, *mk_zeros())
    jax.block_until_ready(warm)

    zbufs = [mk_zeros() for _ in range(iters)]
    jax.block_until_ready(zbufs)   # finish H2D staging before the clock
    outs = []
    t0 = time.perf_counter()
    for i in range(iters):
        outs.append(sharded(*dev_in, *zbufs[i]))
    jax.block_until_ready(outs)
    t1 = time.perf_counter()
    per_call_ns = (t1 - t0) / iters * 1e9
    return per_call_ns, warm


def bench_device(in_maps, reps: int = 16, iters: int = 20, rounds: int = 3):
    """True per-execution device time via reps differencing.

    Compiles a second NEFF that repeats the computation `reps` times
    back-to-back on-device (one TileContext per rep).  The per-dispatch
    axon overhead is identical for both NEFFs, so
    (t_reps - t_1) / (reps - 1) isolates the on-device execution time.
    """
    t1 = min(bench(in_maps, iters=iters)[0] for _ in range(rounds))
    ncR = _build_nc(reps=reps)
    tR = min(bench(in_maps, iters=iters, nc=ncR)[0] for _ in range(rounds))
    return (tR - t1) / (reps - 1)


# ------------------------------------------------------------------- kernel
def _prepare(x_hr, x_lr_inpainted, attn_map, x_lr_blurred):
    """Host sharding prep: upsample, unfold, fp8 quantize + relayout."""
    x_hr = np.asarray(x_hr, np.float32)
    x_lr_inpainted = np.asarray(x_lr_inpainted, np.float32)
    attn_map = np.asarray(attn_map, np.float32)
    x_lr_blurred = np.asarray(x_lr_blurred, np.float32)

    blur_hr = _upsample2(x_lr_blurred)                    # (B, C, 512, 512)
    base = _upsample2(x_lr_inpainted)                     # (B, C, 512, 512)

    q_starts = (0, L - LQ)                                # 0 and 1953
    in_maps = []
    hf_cache = {}
    for core in range(N_CORES):
        b, half = core // 2, core % 2
        if b not in hf_cache:
            hfp = np.zeros((MP, CKK), np.float32)
            hfp[:L] = _unfold_hf(x_hr[b], blur_hr[b])
            # [m, d] -> [p, t, i, d] with m = t*256 + i*128 + p
            hq = hfp.reshape(NT, 2, 128, CKK).transpose(2, 0, 1, 3)
            hf_cache[b] = np.ascontiguousarray(
                _fp8(hq).reshape(128, NT * 2 * CKK))
        q0 = q_starts[half]
        ap = np.zeros((LQP, MP), np.float32)
        ap[:LQ, :L] = attn_map[b, 0, q0:q0 + LQ, :] * ATTN_SCALE
        # [qi, m] -> [qt, p, t, i, q] with qi = qt*128 + q, m = t*256+i*128+p
        a5 = ap.reshape(NQT, 128, NT, 2, 128).transpose(0, 4, 2, 3, 1)
        if SWI:
            # HW SwInterleave weight order: per t-chunk, columns reversed and
            # the two k-tiles interleaved per column: col = 2*(127-q) + i.
            a5 = a5[..., ::-1].transpose(0, 1, 2, 4, 3)
        at = np.ascontiguousarray(_fp8(a5).reshape(NQT * 128, NT * 256))
        in_maps.append({"attn_dr": at, "hf_dr": hf_cache[b]})
    return in_maps, base


def _finish(per_core_rec, base):
    """Gather: rescale, stitch q-halves, fold, normalize, add base."""
    inv = 1.0 / ATTN_SCALE
    cols = np.empty((B, CKK, L), np.float32)
    for b in range(B):
        rec_a = per_core_rec[2 * b]                       # (2048, 768)
        rec_b = per_core_rec[2 * b + 1]
        cols[b, :, :LQ] = rec_a[:LQ].T * inv
        cols[b, :, LQ:] = rec_b[2 * LQ - L:LQ].T * inv
    img = _fold(cols)
    out = base + img / _norm_map()
    return out.astype(np.float32)


def kernel(x_hr, x_lr_inpainted, attn_map, x_lr_blurred):
    global LAST_RESULT
    from concourse.bass_utils import run_bass_kernel_spmd

    in_maps, base = _prepare(x_hr, x_lr_inpainted, attn_map, x_lr_blurred)
    nc = _get_nc()
    trace = bool(os.environ.get("KERNEL_TRACE"))
    res = run_bass_kernel_spmd(nc, in_maps, list(range(N_CORES)), trace=trace)
    LAST_RESULT = res
    return _finish([res.results[c]["rec"] for c in range(N_CORES)], base)
